# revision 1
# baseline (speedup 1.0000x reference)
"""Trainium2 Bass kernel for ExpanderLinear: out = x @ (W * mask).T

Shapes (hardcoded): x [8192, 4096] f32, weight [4096, 4096] f32,
mask [4096, 4096] f32 -> out [8192, 4096] f32.

Sharding: tensor-parallel over output features. Core c gets rows
[c*512, (c+1)*512) of weight/mask plus the full x, and produces
outT_c = ((W_c * mask_c) @ x.T) as a [512, 8192] tile; the host
transposes and concatenates the 8 tiles.

Per-core kernel (all matmuls in float32r: 1 cycle/row at N>=512,
~1e-4 scale-relative error vs the fp32 reference):
  1. Load W_c/mask_c, mask-multiply on DVE, PE-transpose to build
     WmT [4096(i), 512(o)] in SBUF as float32r.
  2. For each 512-row batch chunk: DMA x rows, PE-transpose to
     xT [4096(i), 512(b)] (float32r via the PSUM->SBUF DVE copy),
     then accumulate psum[o:128, b:512] over the 32 K-chunks with
     lhsT=WmT chunk (weights), rhs=xT chunk.
"""

import numpy as np

import concourse.bass as bass
import concourse.mybir as mybir
import concourse.tile as tile
from concourse import bacc
from concourse.bass_utils import run_bass_kernel_spmd
from concourse.masks import make_identity

P = 128
D_IN = 4096
D_OUT = 4096
BATCH = 8192
N_CORES = 8
O_PER_CORE = D_OUT // N_CORES  # 512
KC = D_IN // P  # 32 contraction chunks
B_CHUNK = 512
N_BCHUNK = BATCH // B_CHUNK  # 16
OT = O_PER_CORE // P  # 4 output partition tiles per core

F32 = mybir.dt.float32
F32R = mybir.dt.float32r


def build_nc():
    nc = bacc.Bacc("TRN2", target_bir_lowering=False, debug=False, num_devices=N_CORES)

    x_d = nc.dram_tensor("x", [BATCH, D_IN], F32, kind="ExternalInput")
    w_d = nc.dram_tensor("w", [O_PER_CORE, D_IN], F32, kind="ExternalInput")
    m_d = nc.dram_tensor("mask", [O_PER_CORE, D_IN], F32, kind="ExternalInput")
    outT_d = nc.dram_tensor("outT", [O_PER_CORE, BATCH], F32, kind="ExternalOutput")

    with tile.TileContext(nc) as tc:
        with (
            tc.tile_pool(name="persist", bufs=1) as persist,
            tc.tile_pool(name="stage", bufs=3) as stage,
            tc.tile_pool(name="outp", bufs=3) as outp,
            tc.tile_pool(name="tpsum", bufs=3, space="PSUM") as tpsum,
            tc.tile_pool(name="mpsum", bufs=4, space="PSUM") as mpsum,
        ):
            ident = persist.tile([P, P], F32)
            make_identity(nc, ident)

            # WmT[i, o] for this core's o-slice: [128, KC, O_PER_CORE] f32r
            wmT = persist.tile([P, KC, O_PER_CORE], F32R)
            for ot in range(OT):
                w_t = stage.tile([P, D_IN], F32, tag="stage")
                m_t = stage.tile([P, D_IN], F32, tag="stage")
                nc.sync.dma_start(w_t, w_d[ot * P : (ot + 1) * P, :])
                nc.sync.dma_start(m_t, m_d[ot * P : (ot + 1) * P, :])
                wm = stage.tile([P, D_IN], F32, tag="stage")
                nc.vector.tensor_mul(wm, w_t, m_t)
                for g in range(KC // 4):
                    psT = tpsum.tile([P, 512], F32)
                    for k in range(4):
                        ic = 4 * g + k
                        nc.tensor.matmul(
                            psT[:, k * P : (k + 1) * P],
                            wm[:, ic * P : (ic + 1) * P],
                            ident,
                            is_transpose=True,
                            start=(k == 0),
                            stop=(k == 3),
                        )
                    # psT free axis = (ic_sub, o%128); scatter into wmT
                    nc.vector.tensor_copy(
                        wmT[:, 4 * g : 4 * g + 4, ot * P : (ot + 1) * P], psT
                    )

            # xT for one batch chunk: [128, KC, B_CHUNK] f32r
            xT = persist.tile([P, KC, B_CHUNK], F32R)
            for bc in range(N_BCHUNK):
                for sub in range(B_CHUNK // P):
                    xs = stage.tile([P, D_IN], F32, tag="stage")
                    b0 = bc * B_CHUNK + sub * P
                    nc.sync.dma_start(xs, x_d[b0 : b0 + P, :])
                    for g in range(KC // 4):
                        psT = tpsum.tile([P, 512], F32)
                        for k in range(4):
                            ic = 4 * g + k
                            nc.tensor.matmul(
                                psT[:, k * P : (k + 1) * P],
                                xs[:, ic * P : (ic + 1) * P],
                                ident,
                                is_transpose=True,
                                start=(k == 0),
                                stop=(k == 3),
                            )
                        nc.vector.tensor_copy(
                            xT[:, 4 * g : 4 * g + 4, sub * P : (sub + 1) * P], psT
                        )
                for oc in range(OT):
                    ps = mpsum.tile([P, B_CHUNK], F32)
                    for ic in range(KC):
                        nc.tensor.matmul(
                            ps,
                            wmT[:, ic, oc * P : (oc + 1) * P],
                            xT[:, ic, :],
                            start=(ic == 0),
                            stop=(ic == KC - 1),
                        )
                    ob = outp.tile([P, B_CHUNK], F32)
                    nc.scalar.copy(ob, ps)
                    nc.sync.dma_start(
                        outT_d[oc * P : (oc + 1) * P, bc * B_CHUNK : (bc + 1) * B_CHUNK],
                        ob,
                    )

    nc.compile()
    return nc


_NC_CACHE = None


def kernel(x, weight, mask):
    global _NC_CACHE
    x = np.ascontiguousarray(np.asarray(x, dtype=np.float32))
    weight = np.ascontiguousarray(np.asarray(weight, dtype=np.float32))
    mask = np.ascontiguousarray(np.asarray(mask, dtype=np.float32))
    assert x.shape == (BATCH, D_IN)
    assert weight.shape == (D_OUT, D_IN)
    assert mask.shape == (D_OUT, D_IN)

    if _NC_CACHE is None:
        _NC_CACHE = build_nc()
    nc = _NC_CACHE

    in_maps = []
    for c in range(N_CORES):
        sl = slice(c * O_PER_CORE, (c + 1) * O_PER_CORE)
        in_maps.append(
            {
                "x": x,
                "w": np.ascontiguousarray(weight[sl]),
                "mask": np.ascontiguousarray(mask[sl]),
            }
        )

    res = run_bass_kernel_spmd(nc, in_maps, core_ids=list(range(N_CORES)))

    out = np.empty((BATCH, D_OUT), dtype=np.float32)
    for c in range(N_CORES):
        sl = slice(c * O_PER_CORE, (c + 1) * O_PER_CORE)
        out[:, sl] = res.results[c]["outT"].T
    return out


# revision 5
# speedup vs baseline: 1.4438x; 1.4438x over previous
"""Trainium2 Bass kernel for ExpanderLinear: out = x @ (W * mask).T

Shapes (hardcoded): x [8192, 4096] f32, weight [4096, 4096] f32,
mask [4096, 4096] f32 -> out [8192, 4096] f32.

Strategy: tensor-parallel over output features across 8 cores. The host
pre-transposes the operands (input marshalling, like GEMM pre-packing):
  xT [4096, 8192], wT/maskT column slices [4096, 512] per core.
Each core computes outT_c = (W_c*mask_c) @ x.T as [512, 8192]; the host
transposes/concatenates.

Per-core device kernel (float32r matmuls: 1 cycle/row at N=512,
~1.5e-4 scale-relative error):
  - wmT = round_f32r(wT_c * maskT_c) on DVE -> [128, 32, 512] SBUF.
  - per 512-col chunk of xT: DMA -> SBUF, DVE round to f32r sub-tiles,
    then 4 x 32 accumulating matmuls into psum [128 o, 512 b],
    lhsT = wmT chunk (stationary), rhs = xT chunk (moving).
No PE transposes: the tensor engine runs matmuls only.
"""

import numpy as np

import concourse.bass as bass
import concourse.mybir as mybir
import concourse.tile as tile
from concourse import bacc
from concourse.bass_utils import run_bass_kernel_spmd

P = 128
D_IN = 4096
D_OUT = 4096
BATCH = 8192
N_CORES = 8
O_PER_CORE = D_OUT // N_CORES  # 512
KC = D_IN // P  # 32 contraction chunks
B_CHUNK = 512
N_BCHUNK = BATCH // B_CHUNK  # 16
OT = O_PER_CORE // P  # 4 output partition tiles
KG = 4  # ic groups per chunk
KCG = KC // KG  # 8 ics per group

F32 = mybir.dt.float32
F32R = mybir.dt.float32r


def build_nc():
    nc = bacc.Bacc("TRN2", target_bir_lowering=False, debug=False, num_devices=N_CORES)

    xT_d = nc.dram_tensor("xT", [D_IN, BATCH], F32, kind="ExternalInput")
    wT_d = nc.dram_tensor("wT", [D_IN, O_PER_CORE], F32, kind="ExternalInput")
    mT_d = nc.dram_tensor("maskT", [D_IN, O_PER_CORE], F32, kind="ExternalInput")
    outT_d = nc.dram_tensor("outT", [O_PER_CORE, BATCH], F32, kind="ExternalOutput")

    with tile.TileContext(nc) as tc:
        with (
            tc.tile_pool(name="persist", bufs=1) as persist,
            tc.tile_pool(name="stage", bufs=3) as stage,
            tc.tile_pool(name="xr", bufs=KG) as xrpool,
            tc.tile_pool(name="outp", bufs=2) as outp,
            tc.tile_pool(name="mpsum", bufs=8, space="PSUM") as mpsum,
        ):
            # --- WmT prep: [128, KC, 512] f32r ---
            wmT = persist.tile([P, KC, O_PER_CORE], F32R)
            for h in range(KG):  # quarters to keep staging small
                kc_sl = slice(h * KCG, (h + 1) * KCG)
                r_sl = slice(h * (D_IN // KG), (h + 1) * (D_IN // KG))
                w_t = stage.tile([P, KCG, O_PER_CORE], F32, tag="s")
                m_t = stage.tile([P, KCG, O_PER_CORE], F32, tag="s")
                nc.sync.dma_start(
                    w_t, wT_d[r_sl, :].rearrange("(kc p) o -> p kc o", p=P)
                )
                nc.sync.dma_start(
                    m_t, mT_d[r_sl, :].rearrange("(kc p) o -> p kc o", p=P)
                )
                # mask-multiply with f32r rounding fused into the output dtype
                nc.vector.tensor_mul(wmT[:, kc_sl, :], w_t, m_t)

            # --- main loop over batch chunks ---
            for bc in range(N_BCHUNK):
                xr_subs = []
                for g in range(KG):
                    xs = stage.tile([P, KCG, B_CHUNK], F32, tag="s")
                    rows = slice(g * (D_IN // KG), (g + 1) * (D_IN // KG))
                    cols = slice(bc * B_CHUNK, (bc + 1) * B_CHUNK)
                    nc.sync.dma_start(
                        xs, xT_d[rows, cols].rearrange("(kc p) b -> p kc b", p=P)
                    )
                    xr = xrpool.tile([P, KCG, B_CHUNK], F32R, tag="xr")
                    nc.vector.tensor_copy(xr, xs)  # f32r rounding
                    xr_subs.append(xr)

                psums = [
                    mpsum.tile([P, B_CHUNK], F32, name=f"ps{oc}", tag="ps")
                    for oc in range(OT)
                ]
                for g in range(KG):
                    for k in range(KCG):
                        ic = g * KCG + k
                        for oc in range(OT):
                            nc.tensor.matmul(
                                psums[oc],
                                wmT[:, ic, oc * P : (oc + 1) * P],
                                xr_subs[g][:, k, :],
                                start=(ic == 0),
                                stop=(ic == KC - 1),
                            )
                for oc in range(OT):
                    ob = outp.tile([P, B_CHUNK], F32)
                    nc.scalar.copy(ob, psums[oc])
                    nc.sync.dma_start(
                        outT_d[
                            oc * P : (oc + 1) * P, bc * B_CHUNK : (bc + 1) * B_CHUNK
                        ],
                        ob,
                    )

    nc.compile()
    return nc


_NC_CACHE = None


def _shard_inputs(x, weight, mask):
    """Host-side marshalling: transpose operands and slice per core."""
    x = np.asarray(x, dtype=np.float32)
    weight = np.asarray(weight, dtype=np.float32)
    mask = np.asarray(mask, dtype=np.float32)
    xT = np.ascontiguousarray(x.T)
    wT = weight.T
    mT = mask.T
    in_maps = []
    for c in range(N_CORES):
        sl = slice(c * O_PER_CORE, (c + 1) * O_PER_CORE)
        in_maps.append(
            {
                "xT": xT,
                "wT": np.ascontiguousarray(wT[:, sl]),
                "maskT": np.ascontiguousarray(mT[:, sl]),
            }
        )
    return in_maps


def kernel(x, weight, mask):
    global _NC_CACHE
    if _NC_CACHE is None:
        _NC_CACHE = build_nc()
    nc = _NC_CACHE

    in_maps = _shard_inputs(x, weight, mask)
    res = run_bass_kernel_spmd(nc, in_maps, core_ids=list(range(N_CORES)))

    out = np.empty((BATCH, D_OUT), dtype=np.float32)
    for c in range(N_CORES):
        sl = slice(c * O_PER_CORE, (c + 1) * O_PER_CORE)
        out[:, sl] = res.results[c]["outT"].T
    return out


# revision 6
# speedup vs baseline: 1.5626x; 1.0823x over previous
"""Trainium2 Bass kernel for ExpanderLinear: out = x @ (W * mask).T

Shapes (hardcoded): x [8192, 4096] f32, weight [4096, 4096] f32,
mask [4096, 4096] f32 -> out [8192, 4096] f32.

Strategy: tensor-parallel over output features across 8 cores. The host
pre-transposes the operands (input marshalling, like GEMM pre-packing):
  xT [4096, 8192], wT/maskT column slices [4096, 512] per core.
Each core computes outT_c = (W_c*mask_c) @ x.T as [512, 8192]; the host
transposes/concatenates.

Per-core device kernel (float32r matmuls: 1 cycle/row at N=512,
~1.5e-4 scale-relative error):
  - wmT = round_f32r(wT_c * maskT_c) on DVE -> [128, 32, 512] SBUF.
  - per 512-col chunk of xT: DMA -> SBUF, DVE round to f32r sub-tiles,
    then 4 x 32 accumulating matmuls into psum [128 o, 512 b],
    lhsT = wmT chunk (stationary), rhs = xT chunk (moving).
No PE transposes: the tensor engine runs matmuls only.
"""

import numpy as np

import concourse.bass as bass
import concourse.mybir as mybir
import concourse.tile as tile
from concourse import bacc
from concourse.bass_utils import run_bass_kernel_spmd

P = 128
D_IN = 4096
D_OUT = 4096
BATCH = 8192
N_CORES = 8
O_PER_CORE = D_OUT // N_CORES  # 512
KC = D_IN // P  # 32 contraction chunks
B_CHUNK = 512
N_BCHUNK = BATCH // B_CHUNK  # 16
OT = O_PER_CORE // P  # 4 output partition tiles
KG = 4  # ic groups per chunk
KCG = KC // KG  # 8 ics per group

F32 = mybir.dt.float32
F32R = mybir.dt.float32r


def build_nc():
    nc = bacc.Bacc("TRN2", target_bir_lowering=False, debug=False, num_devices=N_CORES)

    xT_d = nc.dram_tensor("xT", [D_IN, BATCH], F32, kind="ExternalInput")
    wT_d = nc.dram_tensor("wT", [D_IN, O_PER_CORE], F32, kind="ExternalInput")
    mT_d = nc.dram_tensor("maskT", [D_IN, O_PER_CORE], F32, kind="ExternalInput")
    outT_d = nc.dram_tensor("outT", [O_PER_CORE, BATCH], F32, kind="ExternalOutput")

    with tile.TileContext(nc) as tc:
        with (
            tc.tile_pool(name="persist", bufs=1) as persist,
            tc.tile_pool(name="stage", bufs=3) as stage,
            tc.tile_pool(name="xr", bufs=KG) as xrpool,
            tc.tile_pool(name="outp", bufs=2) as outp,
            tc.tile_pool(name="mpsum", bufs=8, space="PSUM") as mpsum,
        ):
            # --- WmT prep: 4 quarter tiles [128, KCG, 512] f32r, interleaved
            # with bc0's x loads so the first matmul starts early ---
            wmT_q = []

            def emit_wm_quarter(h):
                r_sl = slice(h * (D_IN // KG), (h + 1) * (D_IN // KG))
                w_t = stage.tile([P, KCG, O_PER_CORE], F32, tag="s", name=f"w{h}")
                m_t = stage.tile([P, KCG, O_PER_CORE], F32, tag="s", name=f"m{h}")
                nc.sync.dma_start(
                    w_t, wT_d[r_sl, :].rearrange("(kc p) o -> p kc o", p=P)
                )
                nc.sync.dma_start(
                    m_t, mT_d[r_sl, :].rearrange("(kc p) o -> p kc o", p=P)
                )
                wm = persist.tile([P, KCG, O_PER_CORE], F32R, name=f"wmT{h}")
                # mask-multiply with f32r rounding fused into the output dtype
                nc.vector.tensor_mul(wm, w_t, m_t)
                wmT_q.append(wm)

            def emit_x_sub(bc, g):
                xs = stage.tile([P, KCG, B_CHUNK], F32, tag="s", name="xs")
                rows = slice(g * (D_IN // KG), (g + 1) * (D_IN // KG))
                cols = slice(bc * B_CHUNK, (bc + 1) * B_CHUNK)
                nc.sync.dma_start(
                    xs, xT_d[rows, cols].rearrange("(kc p) b -> p kc b", p=P)
                )
                xr = xrpool.tile([P, KCG, B_CHUNK], F32R, tag="xr", name="xr")
                nc.vector.tensor_copy(xr, xs)  # f32r rounding
                return xr

            emit_wm_quarter(0)
            pending = [emit_x_sub(0, g) for g in range(KG)]
            for h in range(1, KG):
                emit_wm_quarter(h)

            # --- main loop over batch chunks ---
            for bc in range(N_BCHUNK):
                xr_subs = pending
                psums = [
                    mpsum.tile([P, B_CHUNK], F32, name=f"ps{oc}", tag="ps")
                    for oc in range(OT)
                ]
                for g in range(KG):
                    for k in range(KCG):
                        ic = g * KCG + k
                        for oc in range(OT):
                            nc.tensor.matmul(
                                psums[oc],
                                wmT_q[g][:, k, oc * P : (oc + 1) * P],
                                xr_subs[g][:, k, :],
                                start=(ic == 0),
                                stop=(ic == KC - 1),
                            )
                if bc + 1 < N_BCHUNK:
                    pending = [emit_x_sub(bc + 1, g) for g in range(KG)]
                for oc in range(OT):
                    ob = outp.tile([P, B_CHUNK], F32)
                    nc.vector.tensor_copy(ob, psums[oc])
                    nc.sync.dma_start(
                        outT_d[
                            oc * P : (oc + 1) * P, bc * B_CHUNK : (bc + 1) * B_CHUNK
                        ],
                        ob,
                    )

    nc.compile()
    return nc


_NC_CACHE = None


def _shard_inputs(x, weight, mask):
    """Host-side marshalling: transpose operands and slice per core."""
    x = np.asarray(x, dtype=np.float32)
    weight = np.asarray(weight, dtype=np.float32)
    mask = np.asarray(mask, dtype=np.float32)
    xT = np.ascontiguousarray(x.T)
    wT = weight.T
    mT = mask.T
    in_maps = []
    for c in range(N_CORES):
        sl = slice(c * O_PER_CORE, (c + 1) * O_PER_CORE)
        in_maps.append(
            {
                "xT": xT,
                "wT": np.ascontiguousarray(wT[:, sl]),
                "maskT": np.ascontiguousarray(mT[:, sl]),
            }
        )
    return in_maps


def kernel(x, weight, mask):
    global _NC_CACHE
    if _NC_CACHE is None:
        _NC_CACHE = build_nc()
    nc = _NC_CACHE

    in_maps = _shard_inputs(x, weight, mask)
    res = run_bass_kernel_spmd(nc, in_maps, core_ids=list(range(N_CORES)))

    out = np.empty((BATCH, D_OUT), dtype=np.float32)
    for c in range(N_CORES):
        sl = slice(c * O_PER_CORE, (c + 1) * O_PER_CORE)
        out[:, sl] = res.results[c]["outT"].T
    return out


# revision 7
# speedup vs baseline: 1.5786x; 1.0102x over previous
"""Trainium2 Bass kernel for ExpanderLinear: out = x @ (W * mask).T

Shapes (hardcoded): x [8192, 4096] f32, weight [4096, 4096] f32,
mask [4096, 4096] f32 -> out [8192, 4096] f32.

Strategy: tensor-parallel over output features across 8 cores. The host
pre-transposes the operands (input marshalling, like GEMM pre-packing):
  xT [4096, 8192], wT/maskT column slices [4096, 512] per core.
Each core computes outT_c = (W_c*mask_c) @ x.T as [512, 8192]; the host
transposes/concatenates.

Per-core device kernel (float32r matmuls: 1 cycle/row at N=512,
~1.5e-4 scale-relative error):
  - wmT = round_f32r(wT_c * maskT_c) on DVE -> [128, 32, 512] SBUF.
  - per 512-col chunk of xT: DMA -> SBUF, DVE round to f32r sub-tiles,
    then 4 x 32 accumulating matmuls into psum [128 o, 512 b],
    lhsT = wmT chunk (stationary), rhs = xT chunk (moving).
No PE transposes: the tensor engine runs matmuls only.
"""

import numpy as np

import concourse.bass as bass
import concourse.mybir as mybir
import concourse.tile as tile
from concourse import bacc
from concourse.bass_utils import run_bass_kernel_spmd

P = 128
D_IN = 4096
D_OUT = 4096
BATCH = 8192
N_CORES = 8
O_PER_CORE = D_OUT // N_CORES  # 512
KC = D_IN // P  # 32 contraction chunks
B_CHUNK = 512
N_BCHUNK = BATCH // B_CHUNK  # 16
OT = O_PER_CORE // P  # 4 output partition tiles
KG = 4  # ic groups per chunk
KCG = KC // KG  # 8 ics per group

F32 = mybir.dt.float32
F32R = mybir.dt.float32r


def build_nc():
    nc = bacc.Bacc("TRN2", target_bir_lowering=False, debug=False, num_devices=N_CORES)

    xT_d = nc.dram_tensor("xT", [D_IN, BATCH], F32, kind="ExternalInput")
    wT_d = nc.dram_tensor("wT", [D_IN, O_PER_CORE], F32, kind="ExternalInput")
    mT_d = nc.dram_tensor("maskT", [D_IN, O_PER_CORE], F32, kind="ExternalInput")
    outT_d = nc.dram_tensor("outT", [O_PER_CORE, BATCH], F32, kind="ExternalOutput")

    with tile.TileContext(nc) as tc:
        with (
            tc.tile_pool(name="persist", bufs=1) as persist,
            tc.tile_pool(name="stage", bufs=3) as stage,
            tc.tile_pool(name="xr", bufs=KG) as xrpool,
            tc.tile_pool(name="outp", bufs=2) as outp,
            tc.tile_pool(name="mpsum", bufs=8, space="PSUM") as mpsum,
        ):
            # --- WmT prep: 4 quarter tiles [128, KCG, 512] f32r, interleaved
            # with bc0's x loads so the first matmul starts early ---
            wmT_q = []

            def emit_wm_quarter(h):
                r_sl = slice(h * (D_IN // KG), (h + 1) * (D_IN // KG))
                w_t = stage.tile([P, KCG, O_PER_CORE], F32, tag="s", name=f"w{h}")
                m_t = stage.tile([P, KCG, O_PER_CORE], F32, tag="s", name=f"m{h}")
                nc.sync.dma_start(
                    w_t, wT_d[r_sl, :].rearrange("(kc p) o -> p kc o", p=P)
                )
                nc.sync.dma_start(
                    m_t, mT_d[r_sl, :].rearrange("(kc p) o -> p kc o", p=P)
                )
                wm = persist.tile([P, KCG, O_PER_CORE], F32R, name=f"wmT{h}")
                # mask-multiply with f32r rounding fused into the output dtype
                nc.vector.tensor_mul(wm, w_t, m_t)
                wmT_q.append(wm)

            def emit_x_sub(bc, g):
                xs = stage.tile([P, KCG, B_CHUNK], F32, tag="s", name="xs")
                rows = slice(g * (D_IN // KG), (g + 1) * (D_IN // KG))
                cols = slice(bc * B_CHUNK, (bc + 1) * B_CHUNK)
                nc.sync.dma_start(
                    xs, xT_d[rows, cols].rearrange("(kc p) b -> p kc b", p=P)
                )
                xr = xrpool.tile([P, KCG, B_CHUNK], F32R, tag="xr", name="xr")
                nc.vector.tensor_copy(xr, xs)  # f32r rounding
                return xr

            pending = []
            for h in range(KG):
                emit_wm_quarter(h)
                pending.append(emit_x_sub(0, h))

            # --- main loop over batch chunks ---
            for bc in range(N_BCHUNK):
                xr_subs = pending
                psums = [
                    mpsum.tile([P, B_CHUNK], F32, name=f"ps{oc}", tag="ps")
                    for oc in range(OT)
                ]
                for g in range(KG):
                    for k in range(KCG):
                        ic = g * KCG + k
                        for oc in range(OT):
                            nc.tensor.matmul(
                                psums[oc],
                                wmT_q[g][:, k, oc * P : (oc + 1) * P],
                                xr_subs[g][:, k, :],
                                start=(ic == 0),
                                stop=(ic == KC - 1),
                            )
                if bc + 1 < N_BCHUNK:
                    pending = [emit_x_sub(bc + 1, g) for g in range(KG)]
                for oc in range(OT):
                    ob = outp.tile([P, B_CHUNK], F32)
                    nc.vector.tensor_copy(ob, psums[oc])
                    nc.sync.dma_start(
                        outT_d[
                            oc * P : (oc + 1) * P, bc * B_CHUNK : (bc + 1) * B_CHUNK
                        ],
                        ob,
                    )

    nc.compile()
    return nc


_NC_CACHE = None


def _shard_inputs(x, weight, mask):
    """Host-side marshalling: transpose operands and slice per core."""
    x = np.asarray(x, dtype=np.float32)
    weight = np.asarray(weight, dtype=np.float32)
    mask = np.asarray(mask, dtype=np.float32)
    xT = np.ascontiguousarray(x.T)
    wT = weight.T
    mT = mask.T
    in_maps = []
    for c in range(N_CORES):
        sl = slice(c * O_PER_CORE, (c + 1) * O_PER_CORE)
        in_maps.append(
            {
                "xT": xT,
                "wT": np.ascontiguousarray(wT[:, sl]),
                "maskT": np.ascontiguousarray(mT[:, sl]),
            }
        )
    return in_maps


def kernel(x, weight, mask):
    global _NC_CACHE
    if _NC_CACHE is None:
        _NC_CACHE = build_nc()
    nc = _NC_CACHE

    in_maps = _shard_inputs(x, weight, mask)
    res = run_bass_kernel_spmd(nc, in_maps, core_ids=list(range(N_CORES)))

    out = np.empty((BATCH, D_OUT), dtype=np.float32)
    for c in range(N_CORES):
        sl = slice(c * O_PER_CORE, (c + 1) * O_PER_CORE)
        out[:, sl] = res.results[c]["outT"].T
    return out


# revision 8
# speedup vs baseline: 1.5925x; 1.0088x over previous
"""Trainium2 Bass kernel for ExpanderLinear: out = x @ (W * mask).T

Shapes (hardcoded): x [8192, 4096] f32, weight [4096, 4096] f32,
mask [4096, 4096] f32 -> out [8192, 4096] f32.

Strategy: tensor-parallel over output features across 8 cores. The host
pre-transposes the operands (input marshalling, like GEMM pre-packing):
  xT [4096, 8192], wT/maskT column slices [4096, 512] per core.
Each core computes outT_c = (W_c*mask_c) @ x.T as [512, 8192]; the host
transposes/concatenates.

Per-core device kernel (float32r matmuls: 1 cycle/row at N=512,
~1.5e-4 scale-relative error):
  - wmT = round_f32r(wT_c * maskT_c) on DVE -> [128, 32, 512] SBUF.
  - per 512-col chunk of xT: DMA -> SBUF, DVE round to f32r sub-tiles,
    then 4 x 32 accumulating matmuls into psum [128 o, 512 b],
    lhsT = wmT chunk (stationary), rhs = xT chunk (moving).
No PE transposes: the tensor engine runs matmuls only.
"""

import ml_dtypes
import numpy as np

import concourse.bass as bass
import concourse.mybir as mybir
import concourse.tile as tile
from concourse import bacc
from concourse.bass_utils import run_bass_kernel_spmd

P = 128
D_IN = 4096
D_OUT = 4096
BATCH = 8192
N_CORES = 8
O_PER_CORE = D_OUT // N_CORES  # 512
KC = D_IN // P  # 32 contraction chunks
B_CHUNK = 512
N_BCHUNK = BATCH // B_CHUNK  # 16
OT = O_PER_CORE // P  # 4 output partition tiles
KG = 4  # ic groups per chunk
KCG = KC // KG  # 8 ics per group

F32 = mybir.dt.float32
F32R = mybir.dt.float32r
BF16 = mybir.dt.bfloat16


def build_nc():
    nc = bacc.Bacc("TRN2", target_bir_lowering=False, debug=False, num_devices=N_CORES)

    xT_d = nc.dram_tensor("xT", [D_IN, BATCH], F32, kind="ExternalInput")
    wT_d = nc.dram_tensor("wT", [D_IN, O_PER_CORE], F32, kind="ExternalInput")
    mT_d = nc.dram_tensor("maskT", [D_IN, O_PER_CORE], BF16, kind="ExternalInput")
    outT_d = nc.dram_tensor("outT", [O_PER_CORE, BATCH], F32, kind="ExternalOutput")

    with tile.TileContext(nc) as tc:
        with (
            tc.tile_pool(name="persist", bufs=1) as persist,
            tc.tile_pool(name="stage", bufs=3) as stage,
            tc.tile_pool(name="xr", bufs=KG + 1) as xrpool,
            tc.tile_pool(name="outp", bufs=2) as outp,
            tc.tile_pool(name="mpsum", bufs=8, space="PSUM") as mpsum,
        ):
            # --- WmT prep: 4 quarter tiles [128, KCG, 512] f32r, interleaved
            # with bc0's x loads so the first matmul starts early ---
            wmT_q = []

            def emit_wm_quarter(h):
                r_sl = slice(h * (D_IN // KG), (h + 1) * (D_IN // KG))
                w_t = stage.tile([P, KCG, O_PER_CORE], F32, tag="s", name=f"w{h}")
                m_t = stage.tile([P, KCG, O_PER_CORE], BF16, tag="s", name=f"m{h}")
                nc.sync.dma_start(
                    w_t, wT_d[r_sl, :].rearrange("(kc p) o -> p kc o", p=P)
                )
                nc.sync.dma_start(
                    m_t, mT_d[r_sl, :].rearrange("(kc p) o -> p kc o", p=P)
                )
                wm = persist.tile([P, KCG, O_PER_CORE], F32R, name=f"wmT{h}")
                # mask-multiply with f32r rounding fused into the output dtype
                nc.vector.tensor_mul(wm, w_t, m_t)
                wmT_q.append(wm)

            def emit_x_sub(bc, g):
                xs = stage.tile([P, KCG, B_CHUNK], F32, tag="s", name="xs")
                rows = slice(g * (D_IN // KG), (g + 1) * (D_IN // KG))
                cols = slice(bc * B_CHUNK, (bc + 1) * B_CHUNK)
                nc.sync.dma_start(
                    xs, xT_d[rows, cols].rearrange("(kc p) b -> p kc b", p=P)
                )
                xr = xrpool.tile([P, KCG, B_CHUNK], F32R, tag="xr", name="xr")
                nc.vector.tensor_copy(xr, xs)  # f32r rounding
                return xr

            pending = []
            for h in range(KG):
                emit_wm_quarter(h)
                pending.append(emit_x_sub(0, h))

            # --- main loop over batch chunks ---
            for bc in range(N_BCHUNK):
                xr_subs = pending
                psums = [
                    mpsum.tile([P, B_CHUNK], F32, name=f"ps{oc}", tag="ps")
                    for oc in range(OT)
                ]
                for g in range(KG):
                    for k in range(KCG):
                        ic = g * KCG + k
                        for oc in range(OT):
                            nc.tensor.matmul(
                                psums[oc],
                                wmT_q[g][:, k, oc * P : (oc + 1) * P],
                                xr_subs[g][:, k, :],
                                start=(ic == 0),
                                stop=(ic == KC - 1),
                            )
                if bc + 1 < N_BCHUNK:
                    pending = [emit_x_sub(bc + 1, g) for g in range(KG)]
                for oc in range(OT):
                    ob = outp.tile([P, B_CHUNK], F32)
                    nc.vector.tensor_copy(ob, psums[oc])
                    nc.sync.dma_start(
                        outT_d[
                            oc * P : (oc + 1) * P, bc * B_CHUNK : (bc + 1) * B_CHUNK
                        ],
                        ob,
                    )

    nc.compile()
    return nc


_NC_CACHE = None


def _shard_inputs(x, weight, mask):
    """Host-side marshalling: transpose operands and slice per core."""
    x = np.asarray(x, dtype=np.float32)
    weight = np.asarray(weight, dtype=np.float32)
    mask = np.asarray(mask, dtype=np.float32)
    xT = np.ascontiguousarray(x.T)
    wT = weight.T
    mT = mask.T
    in_maps = []
    for c in range(N_CORES):
        sl = slice(c * O_PER_CORE, (c + 1) * O_PER_CORE)
        in_maps.append(
            {
                "xT": xT,
                "wT": np.ascontiguousarray(wT[:, sl]),
                "maskT": np.ascontiguousarray(mT[:, sl]).astype(ml_dtypes.bfloat16),
            }
        )
    return in_maps


def kernel(x, weight, mask):
    global _NC_CACHE
    if _NC_CACHE is None:
        _NC_CACHE = build_nc()
    nc = _NC_CACHE

    in_maps = _shard_inputs(x, weight, mask)
    res = run_bass_kernel_spmd(nc, in_maps, core_ids=list(range(N_CORES)))

    out = np.empty((BATCH, D_OUT), dtype=np.float32)
    for c in range(N_CORES):
        sl = slice(c * O_PER_CORE, (c + 1) * O_PER_CORE)
        out[:, sl] = res.results[c]["outT"].T
    return out
